# revision 3
# baseline (speedup 1.0000x reference)
"""Location-sensitive attention (Tacotron-style) Bass kernel for 8 TRN2 cores.

Full-input contract: kernel(**inputs) takes the unsharded numpy inputs and
returns (attention_context [256,512], attention_weights [256,1024]).

Sharding: pure data parallel over batch B=256 -> 32 per core; all weights
replicated. Inside each core, time t is laid out as t = 8*p + c
(p = SBUF partition 0..127, c = chunk 0..7) so the two big loads
(encoder_output 2MB/batch, processed_encoder_output 512KB/batch) are fully
contiguous per partition.

Math folding (host-side, weights only):
  Wk[k,f]   = sum_c conv_w[c,0,k] * W_loc[f,c]        (31-tap fused conv)
  bias_f[f] = sum_c W_loc[f,c] * conv_b[c]
  pl[b,f]   = lstm_output[b] @ W_lstm.T  (computed on device)
The conv matmul gets a 32nd row of ones whose rhs row carries
(pl[b] + bias_f), so the tanh argument penc + ploc + pl + bias comes out of
a single PSUM accumulation plus one DVE add.
"""

import sys

sys.path.insert(0, "/opt/trn_rl_repo")

import numpy as np

import concourse.bass as bass
import concourse.tile as tile
from concourse import bacc, mybir
from concourse.bass import ts
from concourse.bass_utils import run_bass_kernel_spmd

N_CORES = 8
B_FULL, T, E, F = 256, 1024, 512, 128
B = B_FULL // N_CORES  # 32 per core
C = 8  # chunks per batch; t = 8*p + c
P = 128
KW = 31  # conv taps
PAD = 15

F32 = mybir.dt.float32
F32R = mybir.dt.float32r
AX = mybir.AxisListType
ACT_FN = mybir.ActivationFunctionType

_compiled = None


def _build():
    nc = bacc.Bacc("TRN2", target_bir_lowering=False, debug=False)

    enc = nc.dram_tensor("enc", [B, T, E], F32R, kind="ExternalInput").ap()
    penc = nc.dram_tensor("penc", [B, T, F], F32, kind="ExternalInput").ap()
    awc_pad = nc.dram_tensor("awc_pad", [B, T + KW - 1], F32, kind="ExternalInput").ap()
    lstm_t = nc.dram_tensor("lstm_t", [1024, B], F32, kind="ExternalInput").ap()
    wlstm_t = nc.dram_tensor("wlstm_t", [1024, F], F32, kind="ExternalInput").ap()
    wk_rep = nc.dram_tensor("wk_rep", [KW, B, F], F32, kind="ExternalInput").ap()
    bias_f = nc.dram_tensor("bias_f", [F], F32, kind="ExternalInput").ap()
    we_rep = nc.dram_tensor("we_rep", [C * F], F32, kind="ExternalInput").ap()
    ones_d = nc.dram_tensor("ones_d", [1, T], F32, kind="ExternalInput").ap()

    ctx_out = nc.dram_tensor("ctx_out", [B, E], F32, kind="ExternalOutput").ap()
    aw_out = nc.dram_tensor("aw_out", [B, T], F32, kind="ExternalOutput").ap()

    with tile.TileContext(nc) as tc:
        with (
            tc.tile_pool(name="const", bufs=1) as const,
            tc.tile_pool(name="dram", bufs=1, space="DRAM") as drp,
            tc.tile_pool(name="xsp", bufs=2) as xsp,
            tc.tile_pool(name="pencp", bufs=3) as pencp,
            tc.tile_pool(name="encp", bufs=3) as encp,
            tc.tile_pool(name="combp", bufs=2) as combp,
            tc.tile_pool(name="smalls", bufs=3) as smalls,
            tc.tile_pool(name="ctxp", bufs=2) as ctxp,
            tc.tile_pool(name="ploc_ps", bufs=2, space="PSUM") as ploc_ps,
            tc.tile_pool(name="sums_ps", bufs=2, space="PSUM") as sums_ps,
            tc.tile_pool(name="misc_ps", bufs=2, space="PSUM") as misc_ps,
        ):
            # ---- constants / preamble ----
            wlstm_sb = const.tile([P, 8, F], F32)
            nc.sync.dma_start(
                out=wlstm_sb, in_=wlstm_t.rearrange("(j p) f -> p j f", p=P)
            )
            lstm_sb = const.tile([P, 8, B], F32)
            nc.sync.dma_start(
                out=lstm_sb, in_=lstm_t.rearrange("(j p) b -> p j b", p=P)
            )
            we_sb = const.tile([P, C, F], F32)
            nc.sync.dma_start(
                out=we_sb,
                in_=bass.AP(
                    tensor=we_rep.tensor,
                    offset=we_rep.offset,
                    ap=[[0, P], [F, C], [1, F]],
                ),
            )
            ones_sb = const.tile([P, P], F32)
            nc.vector.memset(ones_sb, 1.0)
            bias_sb = const.tile([B, F], F32)
            nc.sync.dma_start(
                out=bias_sb,
                in_=bass.AP(
                    tensor=bias_f.tensor,
                    offset=bias_f.offset,
                    ap=[[0, B], [1, F]],
                ),
            )

            # wk_ext: [32 partitions = 31 taps + ones-row, B, F]
            wk_ext = const.tile([KW + 1, B, F], F32)
            nc.sync.dma_start(out=wk_ext[0:KW, :, :], in_=wk_rep)

            # pl[b,f] = sum_k lstm_t[k,b] * wlstm_t[k,f]  (+ bias_f)
            pl_psum = misc_ps.tile([B, F], F32, tag="mpsum")
            for j in range(8):
                nc.tensor.matmul(
                    pl_psum,
                    lstm_sb[:, j, :],
                    wlstm_sb[:, j, :],
                    start=(j == 0),
                    stop=(j == 7),
                )
            pl_sb = const.tile([B, F], F32)
            nc.vector.tensor_add(out=pl_sb, in0=pl_psum, in1=bias_sb)
            pl_dram = drp.tile([B, F], F32)
            nc.sync.dma_start(out=pl_dram, in_=pl_sb)
            # broadcast row: wk_ext partition 31 holds (pl[b]+bias) for each b
            nc.sync.dma_start(out=wk_ext[KW : KW + 1, :, :], in_=pl_dram[None, :, :])

            # ---- per-batch pipeline ----
            for b in range(B):
                # conv lhsT rows: xs[k, m] = awc_pad[b, k + m]; row 31 = ones
                xs = xsp.tile([KW + 1, T], F32)
                nc.sync.dma_start(
                    out=xs[0:KW, :],
                    in_=bass.AP(
                        tensor=awc_pad.tensor,
                        offset=awc_pad.offset + b * (T + KW - 1),
                        ap=[[1, KW], [1, T]],
                    ),
                )
                nc.sync.dma_start(out=xs[KW : KW + 1, :], in_=ones_d)
                # xs viewed with m = 8p + c -> [32, c, p]
                xs_v = xs.rearrange("k (p c) -> k c p", c=C)

                penc_t = pencp.tile([P, C, F], F32)
                nc.sync.dma_start(
                    out=penc_t, in_=penc[b].rearrange("(p c) f -> p c f", c=C)
                )
                enc_t = encp.tile([P, C, E], F32R)
                nc.sync.dma_start(
                    out=enc_t, in_=enc[b].rearrange("(p c) e -> p c e", c=C)
                )

                # ploc + pl + bias via PE: out[p, c, f]
                ploc = ploc_ps.tile([P, C, F], F32)
                for c in range(C):
                    nc.tensor.matmul(
                        ploc[:, c, :],
                        xs_v[:, c, :],
                        wk_ext[:, b, :],
                        start=True,
                        stop=True,
                    )

                comb = combp.tile([P, C, F], F32)
                nc.vector.tensor_add(out=comb, in0=ploc, in1=penc_t)
                nc.scalar.activation(out=comb, in_=comb, func=ACT_FN.Tanh)
                nc.vector.tensor_mul(out=comb, in0=comb, in1=we_sb)
                eng = smalls.tile([P, C], F32)
                nc.vector.reduce_sum(out=eng, in_=comb, axis=AX.X)
                expw = smalls.tile([P, C], F32)
                nc.scalar.activation(out=expw, in_=eng, func=ACT_FN.Exp)

                # softmax denominator, broadcast to all partitions via ones-matmul
                sums = sums_ps.tile([P, C], F32)
                nc.tensor.matmul(sums, ones_sb, expw, start=True, stop=True)
                ssum = smalls.tile([P, 1], F32)
                nc.vector.reduce_sum(out=ssum, in_=sums, axis=AX.X)
                sinv = smalls.tile([P, 1], F32)
                nc.vector.reciprocal(out=sinv, in_=ssum)
                wn = smalls.tile([P, C], F32)
                nc.vector.tensor_scalar_mul(out=wn, in0=expw, scalar1=sinv)
                nc.sync.dma_start(
                    out=aw_out[b].rearrange("(p c) -> p c", c=C), in_=wn
                )
                wn_r = smalls.tile([P, C], F32R)
                nc.vector.tensor_copy(out=wn_r, in_=wn)

                # context: ctx[e] = sum_{p,c} wn[p,c] * enc[p,c,e]
                ctx_psum = misc_ps.tile([1, E], F32, tag="mpsum")
                for c in range(C):
                    nc.tensor.matmul(
                        ctx_psum,
                        wn_r[:, c : c + 1],
                        enc_t[:, c, :],
                        start=(c == 0),
                        stop=(c == C - 1),
                    )
                ctx_sb = ctxp.tile([1, E], F32)
                nc.scalar.copy(out=ctx_sb, in_=ctx_psum)
                nc.sync.dma_start(out=ctx_out[b : b + 1, :], in_=ctx_sb)

    nc.finalize()
    return nc


def _get_compiled():
    global _compiled
    if _compiled is None:
        _compiled = _build()
    return _compiled


def _prep_in_maps(
    encoder_output,
    processed_encoder_output,
    lstm_output,
    attention_weights_cum,
    W_lstm,
    conv_w,
    conv_b,
    W_loc,
    W_e,
):
    f32 = np.float32
    # weight folds (tiny, host-side)
    wk = np.einsum("ck,fc->kf", conv_w[:, 0, :], W_loc).astype(f32)  # [31, F]
    wk_rep = np.ascontiguousarray(
        np.broadcast_to(wk[:, None, :], (KW, B, F))
    ).astype(f32)
    bias_f = (W_loc @ conv_b).astype(f32)  # [F]
    we_rep = np.tile(W_e[0].astype(f32), C)  # [C*F]
    wlstm_t = np.ascontiguousarray(W_lstm.T).astype(f32)  # [1024, F]
    ones_d = np.ones((1, T), dtype=f32)

    awc_p = np.zeros((B_FULL, T + KW - 1), dtype=f32)
    awc_p[:, PAD : PAD + T] = attention_weights_cum

    in_maps = []
    for i in range(N_CORES):
        sl = slice(i * B, (i + 1) * B)
        in_maps.append(
            {
                "enc": np.ascontiguousarray(encoder_output[sl]).astype(f32),
                "penc": np.ascontiguousarray(processed_encoder_output[sl]).astype(f32),
                "awc_pad": np.ascontiguousarray(awc_p[sl]),
                "lstm_t": np.ascontiguousarray(lstm_output[sl].T).astype(f32),
                "wlstm_t": wlstm_t,
                "wk_rep": wk_rep,
                "bias_f": bias_f,
                "we_rep": we_rep,
                "ones_d": ones_d,
            }
        )
    return in_maps


def _run(inputs, trace=False, trace_kwargs={}):
    nc = _get_compiled()
    in_maps = _prep_in_maps(**inputs)
    res = run_bass_kernel_spmd(
        nc, in_maps, list(range(N_CORES)), trace=trace, trace_kwargs=trace_kwargs
    )
    ctx = np.concatenate([res.results[i]["ctx_out"] for i in range(N_CORES)], axis=0)
    aw = np.concatenate([res.results[i]["aw_out"] for i in range(N_CORES)], axis=0)
    return (ctx, aw), res


def kernel(**inputs):
    (ctx, aw), _ = _run(inputs, trace=False)
    return ctx, aw
